# revision 2
# baseline (speedup 1.0000x reference)
"""BitNetV3Attention (B=2, S=2048, H=16, DH=128, D=2048) on 8 TRN2 NeuronCores.

Strategy (tensor-parallel over heads + row-parallel o_proj):
  - Each core owns 2 of 16 heads. It computes Q^T/K^T (head-transposed,
    [DH, B*S]) and V ([B*S, DH]) for its heads from the full hidden states
    (replicated read), runs causal flash-style attention per (head, batch),
    producing normalized attn_out^T slices [256, B*S].
  - Two AllToAll collectives (one per local head slot) redistribute attn_out
    from head-sharded to sequence-sharded: core j ends with
    attn_out^T[:, rows_j] for ALL 2048 model dims, where rows_j are 512 rows
    of the [4096, 2048] token matrix. The first A2A fires after local head 0
    finishes and overlaps head 1's attention.
  - Each core computes its 512 output rows against the full Wo. o_proj
    accumulates even d-tiles (from A2A#0) before odd ones (A2A#1) so it can
    start before the second collective lands. Wo slabs prefetch during QKV.

Measured (TimelineSim, calibrated against HW slope timing): 359 us vs
462 us for the fp32r baseline. HW scale-rel max error 4.1e-3 (gate 2e-2).

Scheduling notes:
  - All matmul operands are bf16 (halves HBM/SBUF traffic; PE rate is the
    same as fp32r and partial-width score matmuls stay at 1 cycle/row).
  - QKV inner loop: k-tiles 0-7 k-major (so the second ht slab isn't needed
    until halfway), then k-tiles 8-15 target-major so each PSUM bank is
    released + drained long before the next s-chunk needs it (no PE gap at
    chunk boundaries; gaps reset the PE pstate ramp).
  - Attention is software-pipelined: the score matmul for block i+3 issues
    before the exp-dependent PV/denominator matmuls of block i, hiding the
    ScalarE exp latency.
  - DMA queue discipline: a DMA's semaphore waits hold the issuing engine's
    sequencer, so the SP queue carries only forward-flowing traffic (weights
    k0, ht slabs, Wo slabs, ao writes, output rows) while the two
    collective-dependent a2a_out->SBUF loads sit at the tail of the ACT
    queue where they block nothing.
"""
import sys
for _p in ('/opt/trn_rl_repo', '/root/.axon_site/_ro/trn_rl_repo'):
    if _p not in sys.path:
        sys.path.append(_p)

import numpy as np

import concourse.mybir as mybir
import concourse.tile as tile
from concourse import bacc, bass_utils

B, S, H, DH = 2, 2048, 16, 128
D = H * DH                  # 2048
NS = B * S                  # 4096
NC = 8                      # cores
HL = H // NC                # 2 local heads
DSL = HL * DH               # 256 (d-slice per core)
ROWS = NS // NC             # 512 output rows per core
SCALE = 1.0 / float(np.sqrt(DH))
F32 = mybir.dt.float32
F32R = mybir.dt.float32r
BF16 = mybir.dt.bfloat16
MM_DT = BF16
EXP = mybir.ActivationFunctionType.Exp
NEG = -1.0e30

N_K = D // 128              # 16 contraction tiles
N_SC = NS // 512            # 8 s-chunks for QKV
N_QC = S // 512             # 4 q-chunks per batch


def build_bass(repeat=1):
    nc = bacc.Bacc("TRN2", target_bir_lowering=False, debug=False, num_devices=NC)

    ht = nc.dram_tensor("ht", [D, NS], MM_DT, kind="ExternalInput").ap()
    # Wq/Wk/Wv packed host-side as [D, 3*DSL] so one k-major DMA stream
    # feeds all three projections' k-tiles in consumption order at startup.
    wqkv = nc.dram_tensor("wqkv", [D, 3 * DSL], MM_DT, kind="ExternalInput").ap()
    wot = nc.dram_tensor("wot", [D, D], BF16, kind="ExternalInput").ap()
    pad = nc.dram_tensor("pad", [B, S], F32, kind="ExternalInput").ap()
    tri = nc.dram_tensor("tri", [128, 1024], F32, kind="ExternalInput").ap()
    onesd = nc.dram_tensor("ones", [128, 128], MM_DT, kind="ExternalInput").ap()
    idend = nc.dram_tensor("iden", [128, 128], MM_DT, kind="ExternalInput").ap()
    out = nc.dram_tensor("out", [ROWS, D], F32, kind="ExternalOutput").ap()

    with tile.TileContext(nc) as tc:
        with tc.tile_pool(name="dram", bufs=1, space="DRAM") as dram, \
             tc.tile_pool(name="const", bufs=1) as cpool:
            a2a_in = [dram.tile([NC, DH, 512], BF16, name=f"a2a_in{h}") for h in range(HL)]
            a2a_out = [dram.tile([NC, DH, 512], BF16, name=f"a2a_out{h}") for h in range(HL)]

            tri_sb = cpool.tile([128, 1024], F32)
            pad_sb = cpool.tile([128, B * 16], F32)
            ones_sb = cpool.tile([128, 128], MM_DT)
            iden_sb = cpool.tile([128, 128], MM_DT)

            for _rep in range(repeat):
                _emit_body(nc, tc, a2a_in, a2a_out, tri_sb, pad_sb, ones_sb,
                           iden_sb, ht, wqkv, wot, out,
                           pad, tri, onesd, idend)
    nc.compile()
    return nc


def _emit_consts(nc, tri_sb, pad_sb, ones_sb, iden_sb, pad, tri, onesd, idend):
    # ACT queue, after the first weight chunks: tri/pad/ones are first used
    # at attention (iden goes up front with the k0 weights instead).
    nc.scalar.dma_start(ones_sb[:], onesd)
    nc.scalar.dma_start(tri_sb[:], tri)
    nc.scalar.dma_start(
        pad_sb[:].rearrange("p (b t) -> p b t", b=B),
        pad.rearrange("b (t p) -> p b t", p=128),
    )


def _emit_qkv(nc, tc, qt_sb, kt_sb, v_sb, iden_sb, ht, wqkv,
              first_rep=True, emit_consts_fn=None, iden_dma=None):
    with tc.tile_pool(name="wts", bufs=1) as wpool, \
         tc.tile_pool(name="hts", bufs=1) as hpool, \
         tc.tile_pool(name="vtt", bufs=2) as vpool, \
         tc.tile_pool(name="ps1", bufs=1, space="PSUM") as pp1:
        ht_r = ht.rearrange("(k p) s -> p k s", p=128)
        wsb = wpool.tile([128, N_K * 3 * DSL], MM_DT, name="wqkv")
        woff = {"q": 0, "k": DSL, "v": 2 * DSL}

        def wslice(nm, k, h):
            c = 3 * DSL * k + woff[nm] + 128 * h
            return wsb[:, c:c + 128]

        def w_dma(lo, hi):
            nc.sync.dma_start(
                wsb[:, 3 * DSL * lo:3 * DSL * hi].rearrange(
                    "p (t m) -> p t m", t=hi - lo),
                wqkv[128 * lo:128 * hi, :].rearrange(
                    "(t p) m -> p t m", p=128))

        def new_slab():
            return hpool.tile([128, 8 * 512], MM_DT, tag="ht", bufs=4,
                              name="htslab")

        def slab_dma(slab, sc, half):
            nc.sync.dma_start(
                slab[:].rearrange("p (k s) -> p k s", k=8),
                ht_r[:, 8 * half:8 * half + 8, 512 * sc:512 * sc + 512])

        def slab_piece(slab, half, klo, khi):
            # k indices local to the slab half (global k = 8*half + klocal)
            nc.sync.dma_start(
                slab[:, 512 * klo:512 * khi].rearrange(
                    "p (k s) -> p k s", k=khi - klo),
                ht_r[:, 8 * half + klo:8 * half + khi, 0:512])

        # Startup feed, all on SP, interleaved at per-2-k-tile granularity:
        # each DMA's completion semaphore releases work just ahead of the
        # PE's consumption point, so the pstate ramp never resets.
        slab_pre = {0: [new_slab(), new_slab()], 1: [new_slab(), new_slab()]}
        s0, s1 = slab_pre[0]
        w_dma(0, 1)
        if iden_dma is not None:
            iden_dma()  # tiny; V transposes need it ~14us in
        slab_piece(s0, 0, 0, 2)
        w_dma(1, 2)
        slab_piece(s0, 0, 2, 4)
        w_dma(2, 4)
        slab_piece(s0, 0, 4, 8)
        w_dma(4, 8)
        w_dma(8, 10)
        slab_piece(s1, 1, 0, 2)
        w_dma(10, 12)
        slab_piece(s1, 1, 2, 4)
        w_dma(12, 14)
        slab_piece(s1, 1, 4, 6)
        w_dma(14, 16)
        slab_piece(s1, 1, 6, 8)
        slab_dma(slab_pre[1][0], 1, 0)
        slab_dma(slab_pre[1][1], 1, 1)
        if emit_consts_fn is not None:
            emit_consts_fn()

        # second-half target order: v first so its PE transposes can be
        # emitted behind later matmuls without waiting on the drain copies
        targets = [("v", 0), ("v", 1), ("q", 0), ("q", 1), ("k", 0), ("k", 1)]
        dst = {"q": qt_sb, "k": kt_sb}
        for sc in range(N_SC):
            ps = {(nm, h): pp1.tile([128, 512], F32, tag=f"p{nm}{h}",
                                    name=f"p{nm}{h}")
                  for nm in ("q", "k", "v") for h in range(HL)}
            if sc in slab_pre:
                slabs = slab_pre[sc]
            else:
                slabs = [new_slab(), new_slab()]
                for half in range(2):
                    slab_dma(slabs[half], sc, half)
            # k 0-7: k-major (slab half 1 not needed until halfway)
            for k in range(8):
                htt = slabs[0][:, 512 * k:512 * k + 512]
                for nm, h in targets:
                    nc.tensor.matmul(
                        ps[nm, h][:], wslice(nm, k, h),
                        htt, start=(k == 0), stop=False)
            # k 8-15: target-major; each target's PSUM bank drains while the
            # remaining targets keep the PE busy
            pending_tp = []
            for nm, h in targets:
                for k in range(8, 16):
                    htt = slabs[1][:, 512 * (k - 8):512 * (k - 8) + 512]
                    nc.tensor.matmul(
                        ps[nm, h][:], wslice(nm, k, h),
                        htt, start=False, stop=(k == 15))
                # drain
                if nm == "v":
                    vt = vpool.tile([128, 512], MM_DT, tag=f"vtt{h}",
                                    name=f"vtt{h}")
                    if h == 0:
                        nc.vector.tensor_copy(vt[:], ps[nm, h][:])
                    else:
                        nc.scalar.copy(vt[:], ps[nm, h][:])
                    pending_tp.append((h, vt))
                else:
                    if (nm == "q") == (h == 0):
                        nc.vector.tensor_copy(
                            dst[nm][h][:, 512 * sc:512 * sc + 512], ps[nm, h][:])
                    else:
                        nc.scalar.copy(
                            dst[nm][h][:, 512 * sc:512 * sc + 512], ps[nm, h][:])
                    # one deferred V-transpose batch per non-v target so the
                    # vtt drain has finished by the time the PE reaches it
                    if pending_tp:
                        hv, vt = pending_tp.pop(0)
                        for m in range(4):
                            ptp = pp1.tile([128, 128], MM_DT, tag="ptp",
                                           name="ptp", bufs=2)
                            nc.tensor.transpose(
                                ptp[:], vt[:, 128 * m:128 * m + 128], iden_sb[:])
                            st = 4 * sc + m
                            if (hv + m) % 2 == 0:
                                nc.vector.tensor_copy(
                                    v_sb[hv][:, 128 * st:128 * st + 128], ptp[:])
                            else:
                                nc.scalar.copy(
                                    v_sb[hv][:, 128 * st:128 * st + 128], ptp[:])


def _emit_attention(nc, tc, qt_sb, kt_sb, v_sb, tri_sb, pad_sb, ones_sb,
                    a2a_in, a2a_out):
    blocks = []
    for h in range(HL):
        for b in range(B):
            for qc in range(N_QC):
                n_sk = 4 * qc + 4
                for t in range(n_sk):
                    blocks.append((h, b, qc, t, n_sk))
    N = len(blocks)
    stA = {}      # i -> (ps, o)
    stB = {}      # i -> ex
    qcst = {}     # (h,b,qc) -> (po, pd)
    last_ao = [None]

    with tc.tile_pool(name="att", bufs=1) as apool, \
         tc.tile_pool(name="ps2", bufs=1, space="PSUM") as pp2:

        def emit_A(i):
            h, b, qc, t, n_sk = blocks[i]
            q0 = 512 * qc
            o = max(0, 128 * t - q0)
            ps = pp2.tile([128, 512], F32, tag="ps", bufs=4, name="ps")
            nc.tensor.matmul(
                ps[:, o:512],
                kt_sb[h][:, S * b + 128 * t:S * b + 128 * t + 128],
                qt_sb[h][:, S * b + q0 + o:S * b + q0 + 512],
                start=True, stop=True)
            stA[i] = (ps, o)

        def emit_B(i):
            h, b, qc, t, n_sk = blocks[i]
            ps, o = stA[i]
            if t >= 4 * qc:  # diagonal block: additive causal mask
                nc.vector.tensor_add(
                    ps[:, o:512], ps[:, o:512], tri_sb[:, 512:1024 - o])
            ex = apool.tile([128, 512], MM_DT, tag="ex", bufs=6, name="ex")
            nc.scalar.activation(
                ex[:, o:512], ps[:, o:512], EXP,
                bias=pad_sb[:, 16 * b + t:16 * b + t + 1], scale=SCALE)
            stB[i] = ex
            del stA[i]

        def emit_C(i):
            h, b, qc, t, n_sk = blocks[i]
            o = max(0, 128 * t - 512 * qc)
            ex = stB[i]
            if t == 0:
                po = pp2.tile([128, 512], F32, tag="po", bufs=2, name="po")
                pd = pp2.tile([128, 512], F32, tag="pd", bufs=2, name="pd")
                qcst[(h, b, qc)] = (po, pd)
            po, pd = qcst[(h, b, qc)]
            fl = dict(start=(t == 0), stop=(t == n_sk - 1))
            st = 16 * b + t
            nc.tensor.matmul(
                po[:, o:512], v_sb[h][:, 128 * st:128 * st + 128],
                ex[:, o:512], **fl)
            nc.tensor.matmul(
                pd[:, o:512], ones_sb[:], ex[:, o:512], **fl)
            del stB[i]
            if t == n_sk - 1:
                rec = apool.tile([128, 512], F32, tag="rec", bufs=3, name="rec")
                nc.vector.reciprocal(rec[:], pd[:])
                ao = apool.tile([128, 512], BF16, tag="ao", bufs=4, name="ao")
                nc.vector.tensor_mul(ao[:], po[:], rec[:])
                nc.sync.dma_start(a2a_in[h][4 * b + qc, :, :], ao[:])
                last_ao[0] = ao
                del qcst[(h, b, qc)]
                if b == B - 1 and qc == N_QC - 1:
                    # AllToAll for this head-slot; #0 overlaps head 1's attn
                    nc.gpsimd.collective_compute(
                        "AllToAll", mybir.AluOpType.bypass,
                        replica_groups=[list(range(NC))],
                        ins=[a2a_in[h].opt()], outs=[a2a_out[h].opt()])

        # software pipeline: scores lead exp by 2, exp leads PV by 2
        for i in range(N + 4):
            if i < N:
                emit_A(i)
            if 0 <= i - 2 < N:
                emit_B(i - 2)
            if 0 <= i - 4 < N:
                emit_C(i - 4)
    return last_ao[0]


def _emit_oproj_prefetch(nc, wopool, wot):
    # All 8 Wo slabs (8 MB bf16) prefetch on the SP queue; they sit behind
    # the QKV ht slabs and complete well before attention ends.
    slabs = {}
    wot_r2 = wot.rearrange("(t2 two p) e -> p two t2 e", p=128, two=2)
    for ne in range(4):
        for half in range(2):
            sl = wopool.tile([128, 8 * 512], BF16, tag=f"wo{ne}{half}",
                             name=f"wo{ne}{half}", bufs=1)
            nc.sync.dma_start(
                sl[:].rearrange("p (t e) -> p t e", t=8),
                wot_r2[:, half, :, 512 * ne:512 * ne + 512])
            slabs[(ne, half)] = sl
    return slabs


def _emit_oproj(nc, tc, opool, wo_slabs, obpool, a2a_out, out, last_ao):
    # d-tile g lives at a2a_out[g % 2][g // 2]. Two passes: ALL even-g
    # partial sums (data from A2A#0) accumulate into SBUF staging while
    # A2A#1 is in flight; the odd-g pass then adds on top. An in-order PE
    # never touches an odd tile before the collective lands.
    with tc.tile_pool(name="ps4", bufs=1, space="PSUM") as pp4:
        at_sb = [opool.tile([128, 8 * 512], BF16, name=f"at{half}")
                 for half in range(2)]
        # WAW anchors: 1-element writes into each at chunk, sourced from the
        # last attention output, force the tile scheduler to place these
        # loads after the exp stream (it otherwise hoists the DMAs into the
        # middle of attention, where their collective waits stall the ACT
        # sequencer for ~20us).
        for half in range(2):
            for j in range(8):
                nc.vector.tensor_copy(
                    at_sb[half][0:1, 512 * j:512 * j + 1], last_ao[0:1, 0:1])
        for half in range(2):
            # ACT queue tail: at1 waits on A2A#1's semaphore and must not
            # block the forward-flowing SP traffic. Per-j chunks so the
            # first accumulation matmul starts ~0.4us after the collective
            # lands instead of ~3us.
            for j in range(8):
                nc.scalar.dma_start(
                    at_sb[half][:, 512 * j:512 * j + 512],
                    a2a_out[half][j].rearrange("p s -> p s"))
        evens = [g for g in range(N_K) if g % 2 == 0]
        odds = [g for g in range(N_K) if g % 2 == 1]
        obe = {}
        for ne in range(4):
            for m in range(4):
                pout = pp4.tile([128, 512], F32, tag="pout", name="pout",
                                bufs=6)
                for i, g in enumerate(evens):
                    j = g // 2
                    nc.tensor.matmul(
                        pout[:],
                        at_sb[0][:, 512 * j + 128 * m:512 * j + 128 * m + 128],
                        wo_slabs[(ne, 0)][:, 512 * j:512 * j + 512],
                        start=(i == 0), stop=(i == len(evens) - 1))
                oe = obpool.tile([128, 512], F32, tag="obe", name="obe",
                                 bufs=16)
                nc.vector.tensor_copy(oe[:], pout[:])
                obe[(ne, m)] = oe
        for ne in range(4):
            for m in range(4):
                pout = pp4.tile([128, 512], F32, tag="pout", name="pout",
                                bufs=6)
                for i, g in enumerate(odds):
                    j = g // 2
                    nc.tensor.matmul(
                        pout[:],
                        at_sb[1][:, 512 * j + 128 * m:512 * j + 128 * m + 128],
                        wo_slabs[(ne, 1)][:, 512 * j:512 * j + 512],
                        start=(i == 0), stop=(i == len(odds) - 1))
                ob = obpool.tile([128, 512], F32, tag="ob", name="ob", bufs=4)
                nc.vector.tensor_add(ob[:], pout[:], obe[(ne, m)][:])
                nc.sync.dma_start(
                    out[128 * m:128 * m + 128, 512 * ne:512 * ne + 512], ob[:])


def _emit_body(nc, tc, a2a_in, a2a_out, tri_sb, pad_sb, ones_sb,
               iden_sb, ht, wqkv, wot, out,
               pad, tri, onesd, idend):
    with tc.tile_pool(name="store", bufs=1) as spool:
        qt_sb = [spool.tile([128, NS], MM_DT, name=f"qt{h}") for h in range(HL)]
        kt_sb = [spool.tile([128, NS], MM_DT, name=f"kt{h}") for h in range(HL)]
        v_sb = [spool.tile([128, NS], MM_DT, name=f"v{h}") for h in range(HL)]

        _emit_qkv(nc, tc, qt_sb, kt_sb, v_sb, iden_sb, ht, wqkv,
                  emit_consts_fn=lambda: _emit_consts(
                      nc, tri_sb, pad_sb, ones_sb, iden_sb,
                      pad, tri, onesd, idend),
                  iden_dma=lambda: nc.sync.dma_start(iden_sb[:], idend))

        # o_proj pools open before attention so the Wo slabs + output staging
        # land in SBUF vacated by the QKV weight/ht pools.
        with tc.tile_pool(name="oproj", bufs=1) as opool, \
             tc.tile_pool(name="wo", bufs=1) as wopool, \
             tc.tile_pool(name="ob", bufs=1) as obpool:
            wo_slabs = _emit_oproj_prefetch(nc, wopool, wot)
            last_ao = _emit_attention(nc, tc, qt_sb, kt_sb, v_sb, tri_sb,
                                      pad_sb, ones_sb, a2a_in, a2a_out)
            _emit_oproj(nc, tc, opool, wo_slabs, obpool, a2a_out, out,
                        last_ao)


_NC_CACHE = None


def _get_nc():
    global _NC_CACHE
    if _NC_CACHE is None:
        _NC_CACHE = build_bass()
    return _NC_CACHE


def make_in_maps(hidden_states, attention_mask, Wq, Wk, Wv, Wo):
    import ml_dtypes
    mm_np = np.float32 if MM_DT == F32R else ml_dtypes.bfloat16
    x = np.ascontiguousarray(np.asarray(hidden_states, dtype=np.float32)).reshape(NS, D)
    ht = np.ascontiguousarray(x.T).astype(mm_np)                     # [D, NS]
    wqt = np.ascontiguousarray(np.asarray(Wq, dtype=np.float32).T).astype(mm_np)
    wkt = np.ascontiguousarray(np.asarray(Wk, dtype=np.float32).T).astype(mm_np)
    wvt = np.ascontiguousarray(np.asarray(Wv, dtype=np.float32).T).astype(mm_np)
    wot = np.ascontiguousarray(
        np.asarray(Wo, dtype=np.float32).T).astype(ml_dtypes.bfloat16)
    mask = np.asarray(attention_mask)
    pad = np.where(mask == 0, np.float32(NEG), np.float32(0.0)).astype(np.float32)
    tri = np.where(
        np.arange(1024, dtype=np.int64)[None, :] >= np.arange(128, dtype=np.int64)[:, None] + 512,
        np.float32(0.0), np.float32(NEG)).astype(np.float32)
    ones = np.ones((128, 128), dtype=np.float32)
    iden = np.eye(128, dtype=np.float32)

    in_maps = []
    for c in range(NC):
        sl = slice(DSL * c, DSL * c + DSL)
        in_maps.append({
            "ht": ht,
            "wqkv": np.ascontiguousarray(np.concatenate(
                [wqt[:, sl], wkt[:, sl], wvt[:, sl]], axis=1)),
            "wot": wot,
            "pad": pad,
            "tri": tri,
            "ones": ones.astype(mm_np),
            "iden": iden.astype(mm_np),
        })
    return in_maps


def assemble_output(results):
    rows = np.concatenate([results[c]["out"] for c in range(NC)], axis=0)
    return rows.reshape(B, S, D).astype(np.float32)


def kernel(hidden_states, attention_mask, Wq, Wk, Wv, Wo):
    nc = _get_nc()
    in_maps = make_in_maps(hidden_states, attention_mask, Wq, Wk, Wv, Wo)
    res = bass_utils.run_bass_kernel_spmd(nc, in_maps, core_ids=list(range(NC)))
    return assemble_output(res.results)
